# revision 11
# baseline (speedup 1.0000x reference)
"""BertSelfAttention (with segment-embedding score bias) on 8 trn2 NeuronCores.

Math (reference semantics), per head h:
    q = X @ Wq.T + bq ; k = X @ Wk.T ; v = X @ Wv.T + bv
    scores = (q*s) @ k.T + (q + b_q_s) @ segrep.T + mask ;  s = 1/sqrt(DH)
    out = softmax(scores) @ v

Device formulation: per (batch, head) an augmented 128-deep contraction
    qhat = [q*s ; q + b_q_s],  khat = [k ; segrep]
    scores^T = khat_tile.T @ qhat  (mask folded into the exp bias),
then exp on ACT, and PV computed V-stationary as ctx^T = V.T @ P^T.

Key structure vs a straightforward schedule:
  * PE column tiling (tile_position): the two query-half PV streams of one
    head run CONCURRENTLY on disjoint 64-partition halves of the PE array
    (128x64 mode), halving PV wall time; softmax denominators are computed
    by four concurrent M=1 ones-contractions (128x32 mode) instead of a
    65th stationary column, and recovered per-query via a PE transpose.
  * The whole schedule is paced by the ACT engine's exp stream (the hard
    floor); projections are cut into 512-token sub-chunks and interleaved
    into the PE's slack between score matmuls so they hide under the exp
    shadow instead of stalling it.

Sharding: tensor-parallel over heads; core c owns heads 2c, 2c+1. Each core
reads the full tokens, computes its head-slice of the output; the host
concatenates along the hidden dim. No collectives.
"""

import os
import sys

for _p in ("/opt/trn_rl_repo", "/root/.axon_site/_ro/trn_rl_repo"):
    if os.path.isdir(_p) and _p not in sys.path:
        sys.path.append(_p)

import numpy as np
import ml_dtypes

B, S, NH, DH = 4, 2048, 16, 64
HID = NH * DH          # 1024
T = B * S              # 8192
N_CORES = 8
HPC = NH // N_CORES    # heads per core = 2
DPC = HPC * DH         # out dims per core = 128
SCALE = 1.0 / 8.0      # 1/sqrt(DH)
KT = HID // 128        # 8 contraction tiles for the projections
SKT = S // 128         # 16 key tiles per sequence
PSUB = 512             # projection sub-chunk tokens
NSUB = S // PSUB       # 4 sub-chunks per batch
NG = B * HPC           # 8 head-phases per core

_cache = {}


def _build_program():
    import concourse.bacc as bacc
    import concourse.tile as tile
    from concourse import masks, mybir
    from contextlib import ExitStack

    bf16 = mybir.dt.bfloat16
    f32 = mybir.dt.float32
    Exp = mybir.ActivationFunctionType.Exp
    add = mybir.AluOpType.add
    mult = mybir.AluOpType.mult

    nc = bacc.Bacc("TRN2", target_bir_lowering=False, debug=False,
                   num_devices=N_CORES)
    xb = nc.dram_tensor("xb", [HID, T], bf16, kind="ExternalInput")
    wq = nc.dram_tensor("wq", [HID, DPC], bf16, kind="ExternalInput")
    wk = nc.dram_tensor("wk", [HID, DPC], bf16, kind="ExternalInput")
    wv = nc.dram_tensor("wv", [HID, DPC], bf16, kind="ExternalInput")
    srt = nc.dram_tensor("srt", [128, T], bf16, kind="ExternalInput")
    rb = nc.dram_tensor("rb", [128, 128], f32, kind="ExternalInput")
    bqa = nc.dram_tensor("bqa", [DPC, 1], f32, kind="ExternalInput")
    bqb = nc.dram_tensor("bqb", [DPC, 1], f32, kind="ExternalInput")
    bv = nc.dram_tensor("bv", [DPC, 1], f32, kind="ExternalInput")
    outd = nc.dram_tensor("out", [T, DPC], f32, kind="ExternalOutput")
    # cross-partition bounce for the plain-q half of qhat
    qbounce = nc.dram_tensor("qbounce", [2, B * NSUB, 64, PSUB], bf16)

    with tile.TileContext(nc) as tc, ExitStack() as octx:
        const = octx.enter_context(tc.tile_pool(name="const", bufs=1))
        kh_pool = octx.enter_context(tc.tile_pool(name="khp", bufs=4))
        qh_pool = octx.enter_context(tc.tile_pool(name="qhp", bufs=4))
        v_pool = octx.enter_context(tc.tile_pool(name="vp", bufs=3))
        stage_pool = octx.enter_context(tc.tile_pool(name="stp", bufs=2))
        pt_pool = octx.enter_context(tc.tile_pool(name="ptp", bufs=46))
        xt_pool = octx.enter_context(tc.tile_pool(name="xtp", bufs=16))
        qs_pool = octx.enter_context(tc.tile_pool(name="qsp", bufs=2))
        vt_pool = octx.enter_context(tc.tile_pool(name="vtp", bufs=2))
        ctxs_pool = octx.enter_context(tc.tile_pool(name="ctxsp", bufs=2))
        dencp_pool = octx.enter_context(tc.tile_pool(name="dencp", bufs=2))
        rcp_pool = octx.enter_context(tc.tile_pool(name="rcpp", bufs=10))
        sc_psum = octx.enter_context(
            tc.tile_pool(name="scp", bufs=2, space="PSUM"))
        # den/ctx/proj accumulators are live at disjoint points of a phase:
        # one 2-buffer bank-pool serves all three ([128, 512] f32 = 1 bank)
        acc_psum = octx.enter_context(
            tc.tile_pool(name="accp", bufs=2, space="PSUM"))
        tp_psum = octx.enter_context(
            tc.tile_pool(name="tpp", bufs=2, space="PSUM"))

        # ---- constants ----
        rb_sb = const.tile([128, 128], f32)
        bqa_sb = const.tile([DPC, 1], f32)
        bqb_sb = const.tile([DPC, 1], f32)
        bv_sb = const.tile([DPC, 1], f32)
        ident = const.tile([128, 128], bf16)
        ones = const.tile([128, 32], bf16)
        wq_sb = const.tile([128, KT, DPC], bf16)
        wk_sb = const.tile([128, KT, DPC], bf16)
        wv_sb = const.tile([128, KT, DPC], bf16)
        nc.sync.dma_start(rb_sb[:], rb[:])
        nc.sync.dma_start(bqa_sb[:], bqa[:])
        nc.sync.dma_start(bqb_sb[:], bqb[:])
        nc.sync.dma_start(bv_sb[:], bv[:])
        for w_sb, w in ((wq_sb, wq), (wk_sb, wk), (wv_sb, wv)):
            nc.sync.dma_start(w_sb[:],
                              w.rearrange("(kt p) d -> p kt d", p=128))
        masks.make_identity(nc, ident[:])
        nc.gpsimd.memset(ones[:], 1.0)

        # PE warmup: dense matmuls un-throttle the clock gate while the
        # first input DMAs are in flight.
        wup = sc_psum.tile([128, 1024], f32, tag="sc", name="wup")
        for _ in range(48):
            nc.tensor.matmul(wup[:, 0:128], ident[:], ident[:],
                             start=True, stop=True)

        # ---- per-batch resident tiles (rotating pools) ----
        khs, qhs, vsbs, stages = {}, {}, {}, {}

        def alloc_batch(bb):
            khs[bb] = [kh_pool.tile([128, S], bf16, tag="kh", name=f"kh{bb}_{i}")
                       for i in range(2)]
            qhs[bb] = [qh_pool.tile([128, S], bf16, tag="qh", name=f"qh{bb}_{i}")
                       for i in range(2)]
            vsbs[bb] = v_pool.tile([128, SKT * 128], bf16, tag="vsb",
                                   name=f"vsb{bb}")

        def get_stage(bb):
            if bb not in stages:
                stages[bb] = stage_pool.tile([128, 16 * 128], f32, tag="st",
                                             name=f"st{bb}")
            return stages[bb]

        # ---- projection sub-chunk emitters ----
        xts_cur = {}

        def proj_mm(psum_tile, w_sb, xts):
            for kt in range(KT):
                nc.tensor.matmul(psum_tile[:], w_sb[:, kt, :], xts[kt][:],
                                 start=(kt == 0), stop=(kt == KT - 1))

        def proj_k(bb, s):
            if bb not in khs:
                alloc_batch(bb)
            cs = slice(bb * S + s * PSUB, bb * S + (s + 1) * PSUB)
            lb = slice(s * PSUB, (s + 1) * PSUB)
            xts = []
            for kt in range(KT):
                xt = xt_pool.tile([128, PSUB], bf16, tag="xt", name="xt")
                nc.sync.dma_start(xt[:], xb[kt * 128:(kt + 1) * 128, cs])
                xts.append(xt)
            xts_cur[(bb, s)] = xts
            nc.sync.dma_start(khs[bb][0][64:128, lb], srt[64:128, cs])
            nc.sync.dma_start(khs[bb][1][0:64, lb], srt[0:64, cs])
            kp = acc_psum.tile([128, PSUB], f32, tag="acc", name="kp")
            proj_mm(kp, wk_sb, xts)
            nc.vector.tensor_copy(khs[bb][0][0:64, lb], kp[0:64, :])
            nc.vector.tensor_copy(khs[bb][1][64:128, lb], kp[64:128, :])

        def proj_q(bb, s, pop_xts=False):
            lb = slice(s * PSUB, (s + 1) * PSUB)
            xts = xts_cur.pop((bb, s)) if pop_xts else xts_cur[(bb, s)]
            qp = acc_psum.tile([128, PSUB], f32, tag="acc", name="qp")
            proj_mm(qp, wq_sb, xts)
            # scaled halves in-lane: out = (q + bq) * s  (bqa holds bq)
            nc.vector.tensor_scalar(qhs[bb][0][0:64, lb], qp[0:64, :],
                                    bqa_sb[0:64, 0:1], SCALE, add, mult)
            nc.vector.tensor_scalar(qhs[bb][1][64:128, lb], qp[64:128, :],
                                    bqa_sb[64:128, 0:1], SCALE, add, mult)
            # plain halves (q + bq + b_q_s) bounce через DRAM to flip lanes
            qs = qs_pool.tile([128, PSUB], bf16, tag="qs", name="qs")
            nc.vector.tensor_scalar_add(qs[:], qp[:], bqb_sb[:, 0:1])
            ci = bb * NSUB + s
            nc.sync.dma_start(qbounce[0, ci], qs[0:64, :])
            nc.sync.dma_start(qbounce[1, ci], qs[64:128, :])
            nc.sync.dma_start(qhs[bb][0][64:128, lb], qbounce[0, ci])
            nc.sync.dma_start(qhs[bb][1][0:64, lb], qbounce[1, ci])

        def proj_v(bb, s):
            xts = xts_cur.pop((bb, s), None)
            if xts is None:
                # batch-0 V runs a phase after its K/Q: re-DMA the X slice
                cs = slice(bb * S + s * PSUB, bb * S + (s + 1) * PSUB)
                xts = []
                for kt in range(KT):
                    xt = xt_pool.tile([128, PSUB], bf16, tag="xt", name="xt")
                    nc.sync.dma_start(xt[:], xb[kt * 128:(kt + 1) * 128, cs])
                    xts.append(xt)
            vp = acc_psum.tile([128, PSUB], f32, tag="acc", name="vp")
            proj_mm(vp, wv_sb, xts)
            vt = vt_pool.tile([128, PSUB], bf16, tag="vt", name="vt")
            nc.vector.tensor_scalar_add(vt[:], vp[:], bv_sb[:, 0:1])
            for tt in range(PSUB // 128):
                gt = s * (PSUB // 128) + tt
                vtp = tp_psum.tile([128, 128], bf16, tag="tp", name="vtp")
                nc.tensor.transpose(
                    vtp[:], vt[:, tt * 128:(tt + 1) * 128], ident[:])
                nc.vector.tensor_copy(
                    vsbs[bb][:, gt * 128:(gt + 1) * 128], vtp[:])

        # ---- attention pieces ----
        def scores_exp(g, qh, kt, pts):
            b, hl = divmod(g, 2)
            sp = sc_psum.tile([128, 1024], f32, tag="sc", name="sp")
            ksl = khs[b][hl][:, kt * 128:(kt + 1) * 128]
            for nn in range(2):
                qsl = qhs[b][hl][:, qh * 1024 + nn * 512:
                                 qh * 1024 + (nn + 1) * 512]
                nc.tensor.matmul(sp[:, nn * 512:(nn + 1) * 512], ksl, qsl,
                                 start=True, stop=True)
            pt = pt_pool.tile([128, 1024], bf16, tag="pt", name="pt")
            col = hl * 64 + b * 16 + kt
            nc.scalar.activation(pt[:], sp[:], Exp,
                                 bias=rb_sb[:, col:col + 1], scale=1.0)
            pts[qh][kt] = pt

        def den_block(g, pts, denp):
            """4 concurrent ones-contractions (128x32 col tiling). The
            stationary is 32 ones-columns so each tile fills its full
            32-row quadrant (same cycle count; keeps psum fully defined)."""
            for kt in range(SKT):
                st = (kt == 0)
                sp = (kt == SKT - 1)
                nc.tensor.matmul(denp[0:32, :], ones[:],
                                 pts[0][kt][:, 0:512], start=st, stop=sp,
                                 tile_position=(0, 0), skip_group_check=True)
                nc.tensor.matmul(denp[32:64, :], ones[:],
                                 pts[0][kt][:, 512:1024], start=st, stop=sp,
                                 tile_position=(0, 32), skip_group_check=True)
                nc.tensor.matmul(denp[64:96, :], ones[:],
                                 pts[1][kt][:, 0:512], start=st, stop=sp,
                                 tile_position=(0, 64), skip_group_check=True)
                nc.tensor.matmul(denp[96:128, :], ones[:],
                                 pts[1][kt][:, 512:1024], start=st, stop=sp,
                                 tile_position=(0, 96), skip_group_check=True)

        def den_drain(g, denp, rcps):
            """denp -> sbuf -> 4 PE transposes -> per-query reciprocals."""
            dcp = dencp_pool.tile([128, 512], bf16, tag="dc", name="dcp")
            nc.vector.tensor_copy(dcp[:], denp[:])
            for j in range(4):
                dtp = tp_psum.tile([128, 128], bf16, tag="tp", name="dtp")
                nc.tensor.transpose(dtp[:], dcp[:, j * 128:(j + 1) * 128],
                                    ident[:])
                rcp = rcp_pool.tile([128, 4], f32, tag="rcp", name="rcp")
                nc.vector.reciprocal(
                    rcp[:],
                    dtp[:].rearrange("p (a x) -> p a x", x=32)[:, :, 0:1])
                rcps[j] = rcp

        def pv_half(g, h, pts, ctxp):
            """Paired PV: query-halves qh0/qh1 on PE column halves."""
            b, hl = divmod(g, 2)
            hs = slice(h * 512, (h + 1) * 512)
            for kt in range(SKT):
                vb = vsbs[b][:, kt * 128 + hl * 64:kt * 128 + hl * 64 + 64]
                st = (kt == 0)
                sp = (kt == SKT - 1)
                nc.tensor.matmul(ctxp[0:64, :], vb, pts[0][kt][:, hs],
                                 start=st, stop=sp, tile_position=(0, 0),
                                 skip_group_check=True)
                nc.tensor.matmul(ctxp[64:128, :], vb, pts[1][kt][:, hs],
                                 start=st, stop=sp, tile_position=(0, 64),
                                 skip_group_check=True)

        def ctx_copy(h, ctxp, ctxs):
            nc.vector.tensor_copy(ctxs[:, h * 512:(h + 1) * 512], ctxp[:])

        def norm_quarter(g, half, ctxs, rcps, stage):
            """Transpose ctx^T back per 128-query tile; divide by denom."""
            b, hl = divmod(g, 2)
            for jj in range(half * 4, half * 4 + 4):
                tp = tp_psum.tile([128, 128], bf16, tag="tp", name="tp")
                nc.tensor.transpose(tp[:], ctxs[:, jj * 128:(jj + 1) * 128],
                                    ident[:])
                j = jj % 4
                ch = 0 if jj < 4 else 1
                for item in range(2):
                    gq = item * 8 + jj
                    nc.vector.tensor_scalar_mul(
                        stage[:, gq * 128 + hl * 64:gq * 128 + hl * 64 + 64],
                        tp[:, item * 64:item * 64 + 64],
                        rcps[j][:, 2 * item + ch:2 * item + ch + 1])

        def flush_out(bb):
            nc.sync.dma_start(
                outd[bb * S:(bb + 1) * S, :]
                .rearrange("(gq q) hd -> q gq hd", q=128),
                get_stage(bb)[:].rearrange("q (gq hd) -> q gq hd", hd=DPC))

        # ---- the schedule ----
        # preamble: K and Q projections of batch 0 (paired per sub-chunk so
        # at most two sub-chunks of X tiles are in flight)
        for s in range(NSUB):
            proj_k(0, s)
            proj_q(0, s, pop_xts=True)

        prev = None      # (g, pts) of the previous head-phase
        all_pts = {}

        for g in range(NG):
            b, hl = divmod(g, 2)
            pts = {0: {}, 1: {}}
            all_pts[g] = pts

            # attention drain state for head g-1
            pstate = {}
            if prev is not None:
                pg, ppts = prev
                pb, phl = divmod(pg, 2)
                pstate["denp"] = None
                pstate["rcps"] = {}
                pstate["ctxs"] = ctxs_pool.tile([128, 1024], bf16, tag="cx",
                                                name="ctxs")
                pstate["stage"] = get_stage(pb)

            # projection slots carried by this phase: batch b+1
            pslots = []
            if b + 1 < B:
                s0 = 0 if hl == 0 else 2
                for s in (s0, s0 + 1):
                    pslots.append(lambda s=s: proj_k(b + 1, s))
                    pslots.append(lambda s=s: proj_q(b + 1, s))
                    pslots.append(lambda s=s: proj_v(b + 1, s))
            if g == 0:
                # batch 0 V projections ride in phase 0
                for s in range(NSUB):
                    pslots.append(lambda s=s: proj_v(0, s))

            def interleave(qh, kt):
                if prev is not None:
                    pg, ppts = prev
                    if (qh, kt) == (0, 1):
                        pstate["denp"] = acc_psum.tile([128, 512], f32,
                                                       tag="acc", name="denp")
                        den_block(pg, ppts, pstate["denp"])
                        return
                    if (qh, kt) == (0, 3):
                        den_drain(pg, pstate["denp"], pstate["rcps"])
                        return
                    if (qh, kt) == (0, 5):
                        pstate["ctxp"] = acc_psum.tile([128, 512], f32,
                                                       tag="acc", name="ctxp")
                        pv_half(pg, 0, ppts, pstate["ctxp"])
                        return
                    if (qh, kt) == (0, 7):
                        ctx_copy(0, pstate["ctxp"], pstate["ctxs"])
                        return
                    if (qh, kt) == (0, 9):
                        pstate["ctxp"] = acc_psum.tile([128, 512], f32,
                                                       tag="acc", name="ctxp")
                        pv_half(pg, 1, ppts, pstate["ctxp"])
                        return
                    if (qh, kt) == (0, 11):
                        ctx_copy(1, pstate["ctxp"], pstate["ctxs"])
                        return
                    if (qh, kt) == (0, 13):
                        norm_quarter(pg, 0, pstate["ctxs"], pstate["rcps"],
                                     pstate["stage"])
                        return
                    if (qh, kt) == (0, 15):
                        norm_quarter(pg, 1, pstate["ctxs"], pstate["rcps"],
                                     pstate["stage"])
                        del all_pts[pg]
                        return
                    if (qh, kt) == (1, 1) and pg % 2 == 1:
                        flush_out(pg // 2)
                        return
                # projection slots in remaining gaps
                if qh == 1 and kt % 2 == 1 and pslots:
                    pslots.pop(0)()
                elif prev is None and qh == 0 and kt % 2 == 1 and pslots:
                    pslots.pop(0)()

            for qh in range(2):
                for kt in range(SKT):
                    scores_exp(g, qh, kt, pts)
                    interleave(qh, kt)
            while pslots:
                pslots.pop(0)()
            prev = (g, pts)

        # ---- tail: drain head 7 ----
        g, ppts = prev
        b, hl = divmod(g, 2)
        denp = acc_psum.tile([128, 512], f32, tag="acc", name="denp")
        den_block(g, ppts, denp)
        rcps = {}
        den_drain(g, denp, rcps)
        ctxs = ctxs_pool.tile([128, 1024], bf16, tag="cx", name="ctxs")
        for h in range(2):
            ctxp = acc_psum.tile([128, 512], f32, tag="acc", name="ctxp")
            pv_half(g, h, ppts, ctxp)
            ctx_copy(h, ctxp, ctxs)
        stage = get_stage(b)
        norm_quarter(g, 0, ctxs, rcps, stage)
        norm_quarter(g, 1, ctxs, rcps, stage)
        flush_out(b)

    nc.compile()
    return nc


def get_program():
    if "nc" not in _cache:
        _cache["nc"] = _build_program()
    return _cache["nc"]


def make_in_maps(hidden_states, attention_mask, seg_ids, Wq, bq, Wk, Wv, bv,
                 seg_table, b_q_s):
    """Host-side shard + layout prep. Cheap (weights/bias reshapes, one bf16
    cast of X, 2-row segment gather); all O(T*S) math stays on device."""
    bf = ml_dtypes.bfloat16
    X = np.asarray(hidden_states, np.float32).reshape(T, HID)
    xb = np.ascontiguousarray(X.astype(bf).T)
    m = np.asarray(seg_ids).reshape(T).astype(np.int64)
    mask = np.asarray(attention_mask, np.float32).reshape(B, S)
    st = np.asarray(seg_table, np.float32)              # [2, HID]
    bqs = np.asarray(b_q_s, np.float32).reshape(NH, DH)
    Wq = np.asarray(Wq, np.float32)
    Wk = np.asarray(Wk, np.float32)
    Wv = np.asarray(Wv, np.float32)
    bq = np.asarray(bq, np.float32)
    bv = np.asarray(bv, np.float32)

    # mask-only per-key bias, same layout for both heads of a core:
    # rb[key, hl*64 + b*16 + kt] = mask[b, kt*128+key]
    rb_half = mask.reshape(B, 16, 128).transpose(2, 0, 1).reshape(128, 64)
    rb_c = np.ascontiguousarray(
        np.concatenate([rb_half, rb_half], axis=1).astype(np.float32))

    in_maps = []
    for c in range(N_CORES):
        h0, h1 = c * HPC, c * HPC + 1
        s0, s1 = slice(h0 * DH, (h0 + 1) * DH), slice(h1 * DH, (h1 + 1) * DH)
        # bias vectors: bqa = bq (scaled path applies (q+bq)*s),
        # bqb = bq + b_q_s (plain path)
        bqa_c = np.concatenate([bq[s0], bq[s1]])
        bqb_c = np.concatenate([bq[s0] + bqs[h0], bq[s1] + bqs[h1]])
        # segrep^T halves: [0:64]=head1, [64:128]=head0
        srt_c = np.empty((128, T), np.float32)
        srt_c[0:64, :] = st[np.ix_(m, range(s1.start, s1.stop))].T
        srt_c[64:128, :] = st[np.ix_(m, range(s0.start, s0.stop))].T
        sl = slice(c * DPC, (c + 1) * DPC)
        in_maps.append({
            "xb": xb,
            "wq": np.ascontiguousarray(Wq[sl, :].T).astype(bf),
            "wk": np.ascontiguousarray(Wk[sl, :].T).astype(bf),
            "wv": np.ascontiguousarray(Wv[sl, :].T).astype(bf),
            "srt": srt_c.astype(bf),
            "rb": rb_c,
            "bqa": np.ascontiguousarray(bqa_c.reshape(DPC, 1)),
            "bqb": np.ascontiguousarray(bqb_c.reshape(DPC, 1)),
            "bv": np.ascontiguousarray(bv[sl].reshape(DPC, 1)),
        })
    return in_maps


def assemble_output(results):
    return np.concatenate(
        [np.asarray(r["out"], np.float32).reshape(B, S, DPC) for r in results],
        axis=2)


def kernel(hidden_states, attention_mask, seg_ids, Wq, bq, Wk, Wv, bv,
           seg_table, b_q_s):
    from concourse.bass_utils import run_bass_kernel_spmd
    nc = get_program()
    in_maps = make_in_maps(hidden_states, attention_mask, seg_ids, Wq, bq,
                           Wk, Wv, bv, seg_table, b_q_s)
    res = run_bass_kernel_spmd(nc, in_maps, list(range(N_CORES)))
    return assemble_output(res.results)


if __name__ == "__main__":
    get_program()
    print("program built + compiled ok")


# revision 13
# speedup vs baseline: 1.1089x; 1.1089x over previous
"""BertSelfAttention (with segment-embedding score bias) on 8 trn2 NeuronCores.

Math (reference semantics), per head h:
    q = X @ Wq.T + bq ; k = X @ Wk.T ; v = X @ Wv.T + bv
    scores = (q*s) @ k.T + (q + b_q_s) @ segrep.T + mask ;  s = 1/sqrt(DH)
    out = softmax(scores) @ v

Device formulation: per (batch, head) an augmented 128-deep contraction
    qhat = [q*s ; q + b_q_s],  khat = [k ; segrep]
    scores^T = khat_tile.T @ qhat  (mask folded into the exp bias); exp on
ACT; PV computed V-stationary as ctx^T = [V|1].T @ P^T (the ones column
accumulates the softmax denominator), then a PE transpose per 128-query
tile and a reciprocal-scaled writeback.

Schedule: the ACT engine's exp stream (~1.04us per [128,1024] tile) is the
hard floor, so everything else hides in its shadow: PV of item g-1
interleaves per key-tile with the scores of item g, and the projections are
cut into 512-token single-matrix slices placed between score tiles so the
score/psum stream (and with it the exp pipeline) never pauses for a long
projection burst. ACT runs nothing but exp; scaling/bias work lives on DVE.

Sharding: tensor-parallel over heads; core c owns heads 2c, 2c+1. Each core
reads the full tokens and computes its head-slice of the output; the host
concatenates along the hidden dim. No collectives.
"""

import os
import sys

for _p in ("/opt/trn_rl_repo", "/root/.axon_site/_ro/trn_rl_repo"):
    if os.path.isdir(_p) and _p not in sys.path:
        sys.path.append(_p)

import numpy as np
import ml_dtypes

B, S, NH, DH = 4, 2048, 16, 64
HID = NH * DH          # 1024
T = B * S              # 8192
N_CORES = 8
HPC = NH // N_CORES    # heads per core = 2
DPC = HPC * DH         # out dims per core = 128
SCALE = 1.0 / 8.0      # 1/sqrt(DH)
KT = HID // 128        # 8 contraction tiles for the projections
SKT = S // 128         # 16 key tiles per sequence
PSUB = 512             # projection slice tokens
NSUB = S // PSUB       # 4 slices per batch
QBLK = 1024            # queries per item
NQT = QBLK // 128      # query tiles per item

_cache = {}


def _build_program():
    import concourse.bacc as bacc
    import concourse.tile as tile
    from concourse import masks, mybir
    from contextlib import ExitStack

    bf16 = mybir.dt.bfloat16
    f32 = mybir.dt.float32
    Exp = mybir.ActivationFunctionType.Exp
    add = mybir.AluOpType.add
    mult = mybir.AluOpType.mult

    nc = bacc.Bacc("TRN2", target_bir_lowering=False, debug=False,
                   num_devices=N_CORES)
    xb = nc.dram_tensor("xb", [HID, T], bf16, kind="ExternalInput")
    wq = nc.dram_tensor("wq", [HID, DPC], bf16, kind="ExternalInput")
    wk = nc.dram_tensor("wk", [HID, DPC], bf16, kind="ExternalInput")
    wv = nc.dram_tensor("wv", [HID, DPC], bf16, kind="ExternalInput")
    srt = nc.dram_tensor("srt", [128, T], bf16, kind="ExternalInput")
    rb = nc.dram_tensor("rb", [128, 128], f32, kind="ExternalInput")
    bqa = nc.dram_tensor("bqa", [DPC, 1], f32, kind="ExternalInput")
    bqb = nc.dram_tensor("bqb", [DPC, 1], f32, kind="ExternalInput")
    bv = nc.dram_tensor("bv", [DPC, 1], f32, kind="ExternalInput")
    outd = nc.dram_tensor("out", [T, DPC], f32, kind="ExternalOutput")
    # cross-partition bounce for the plain-q half of qhat
    qbounce = nc.dram_tensor("qbounce", [2, B * NSUB, 64, PSUB], bf16)

    with tile.TileContext(nc) as tc, ExitStack() as octx:
        const = octx.enter_context(tc.tile_pool(name="const", bufs=1))
        kh_pool = octx.enter_context(tc.tile_pool(name="khp", bufs=4))
        qh_pool = octx.enter_context(tc.tile_pool(name="qhp", bufs=4))
        v_pool = octx.enter_context(tc.tile_pool(name="vp", bufs=3))
        stage_pool = octx.enter_context(tc.tile_pool(name="stp", bufs=2))
        pt_pool = octx.enter_context(tc.tile_pool(name="ptp", bufs=36))
        xt_pool = octx.enter_context(tc.tile_pool(name="xtp", bufs=16))
        qs_pool = octx.enter_context(tc.tile_pool(name="qsp", bufs=2))
        vt_pool = octx.enter_context(tc.tile_pool(name="vtp", bufs=2))
        ctxs_pool = octx.enter_context(tc.tile_pool(name="ctxsp", bufs=2))
        rcp_pool = octx.enter_context(tc.tile_pool(name="rcpp", bufs=8))
        sc_psum = octx.enter_context(
            tc.tile_pool(name="scp", bufs=2, space="PSUM"))
        ctx_psum = octx.enter_context(
            tc.tile_pool(name="ctxp", bufs=1, space="PSUM"))
        tp_psum = octx.enter_context(
            tc.tile_pool(name="tpp", bufs=2, space="PSUM"))

        # ---- constants ----
        rb_sb = const.tile([128, 128], f32)
        bqa_sb = const.tile([DPC, 1], f32)
        bqb_sb = const.tile([DPC, 1], f32)
        bv_sb = const.tile([DPC, 1], f32)
        ident = const.tile([128, 128], bf16)
        wq_sb = const.tile([128, KT, DPC], bf16)
        wk_sb = const.tile([128, KT, DPC], bf16)
        wv_sb = const.tile([128, KT, DPC], bf16)
        nc.sync.dma_start(rb_sb[:], rb[:])
        nc.sync.dma_start(bqa_sb[:], bqa[:])
        nc.sync.dma_start(bqb_sb[:], bqb[:])
        nc.sync.dma_start(bv_sb[:], bv[:])
        for w_sb, w in ((wq_sb, wq), (wk_sb, wk), (wv_sb, wv)):
            nc.sync.dma_start(w_sb[:],
                              w.rearrange("(kt p) d -> p kt d", p=128))
        masks.make_identity(nc, ident[:])

        # PE warmup: dense matmuls un-throttle the clock gate while the
        # first input DMAs are in flight.
        wup = sc_psum.tile([128, 1024], f32, tag="sc", name="wup")
        for _ in range(48):
            nc.tensor.matmul(wup[:, 0:128], ident[:], ident[:],
                             start=True, stop=True)

        # ---- per-batch resident tiles (rotating pools) ----
        khs, qhs, vsbs, stages = {}, {}, {}, {}

        def alloc_batch(bb):
            khs[bb] = [kh_pool.tile([128, S], bf16, tag="kh",
                                    name=f"kh{bb}_{i}") for i in range(2)]
            qhs[bb] = [qh_pool.tile([128, S], bf16, tag="qh",
                                    name=f"qh{bb}_{i}") for i in range(2)]
            v = v_pool.tile([128, SKT * 130], bf16, tag="vsb",
                            name=f"vsb{bb}")
            nc.gpsimd.memset(v[:], 1.0)   # preset denominators' ones cols
            vsbs[bb] = v

        def get_stage(bb):
            if bb not in stages:
                stages[bb] = stage_pool.tile([128, 16 * 128], f32, tag="st",
                                             name=f"st{bb}")
            return stages[bb]

        # ---- projection slices (one matrix x 512 tokens each) ----
        xts_cur = {}

        def proj_mm(psum_tile, w_sb, xts):
            for kt in range(KT):
                nc.tensor.matmul(psum_tile[:, 0:PSUB], w_sb[:, kt, :],
                                 xts[kt][:], start=(kt == 0),
                                 stop=(kt == KT - 1))

        def proj_k(bb, s):
            if bb not in khs:
                alloc_batch(bb)
            cs = slice(bb * S + s * PSUB, bb * S + (s + 1) * PSUB)
            lb = slice(s * PSUB, (s + 1) * PSUB)
            xts = []
            for kt in range(KT):
                xt = xt_pool.tile([128, PSUB], bf16, tag="xt", name="xt")
                nc.sync.dma_start(xt[:], xb[kt * 128:(kt + 1) * 128, cs])
                xts.append(xt)
            xts_cur[(bb, s)] = xts
            nc.sync.dma_start(khs[bb][0][64:128, lb], srt[64:128, cs])
            nc.sync.dma_start(khs[bb][1][0:64, lb], srt[0:64, cs])
            kp = sc_psum.tile([128, 1024], f32, tag="sc", name="kp")
            proj_mm(kp, wk_sb, xts)
            nc.vector.tensor_copy(khs[bb][0][0:64, lb], kp[0:64, 0:PSUB])
            nc.vector.tensor_copy(khs[bb][1][64:128, lb], kp[64:128, 0:PSUB])

        def proj_q(bb, s, pop_xts=False):
            lb = slice(s * PSUB, (s + 1) * PSUB)
            xts = xts_cur.pop((bb, s)) if pop_xts else xts_cur[(bb, s)]
            qp = sc_psum.tile([128, 1024], f32, tag="sc", name="qp")
            proj_mm(qp, wq_sb, xts)
            # scaled halves in-lane: out = (q + bq) * s  (bqa holds bq)
            nc.vector.tensor_scalar(qhs[bb][0][0:64, lb], qp[0:64, 0:PSUB],
                                    bqa_sb[0:64, 0:1], SCALE, add, mult)
            nc.vector.tensor_scalar(qhs[bb][1][64:128, lb],
                                    qp[64:128, 0:PSUB],
                                    bqa_sb[64:128, 0:1], SCALE, add, mult)
            # plain halves (q + bq + b_q_s) bounce through DRAM to flip lanes
            qs = qs_pool.tile([128, PSUB], bf16, tag="qs", name="qs")
            nc.vector.tensor_scalar_add(qs[:], qp[:, 0:PSUB], bqb_sb[:, 0:1])
            ci = bb * NSUB + s
            nc.sync.dma_start(qbounce[0, ci], qs[0:64, :])
            nc.sync.dma_start(qbounce[1, ci], qs[64:128, :])
            nc.sync.dma_start(qhs[bb][0][64:128, lb], qbounce[0, ci])
            nc.sync.dma_start(qhs[bb][1][0:64, lb], qbounce[1, ci])

        def proj_v(bb, s):
            xts = xts_cur.pop((bb, s), None)
            if xts is None:
                # batch-0 V runs an item after its K/Q: re-DMA the X slice
                cs = slice(bb * S + s * PSUB, bb * S + (s + 1) * PSUB)
                xts = []
                for kt in range(KT):
                    xt = xt_pool.tile([128, PSUB], bf16, tag="xt", name="xt")
                    nc.sync.dma_start(xt[:], xb[kt * 128:(kt + 1) * 128, cs])
                    xts.append(xt)
            vp = sc_psum.tile([128, 1024], f32, tag="sc", name="vp")
            proj_mm(vp, wv_sb, xts)
            vt = vt_pool.tile([128, PSUB], bf16, tag="vt", name="vt")
            nc.vector.tensor_scalar_add(vt[:], vp[:, 0:PSUB], bv_sb[:, 0:1])
            for tt in range(PSUB // 128):
                gt = s * (PSUB // 128) + tt
                vtp = tp_psum.tile([128, 128], bf16, tag="tp", name="vtp")
                nc.tensor.transpose(
                    vtp[:], vt[:, tt * 128:(tt + 1) * 128], ident[:])
                nc.vector.tensor_copy(
                    vsbs[bb][:, gt * 130:(gt + 1) * 130]
                    .rearrange("p (h x) -> p h x", h=2)[:, :, 0:64],
                    vtp[:].rearrange("p (h d) -> p h d", h=2))

        # ---- attention pieces ----
        def scores_exp(b, hl, qh, kt, pts):
            sp = sc_psum.tile([128, 1024], f32, tag="sc", name="sp")
            ksl = khs[b][hl][:, kt * 128:(kt + 1) * 128]
            for nn in range(2):
                qsl = qhs[b][hl][:, qh * QBLK + nn * 512:
                                 qh * QBLK + (nn + 1) * 512]
                nc.tensor.matmul(sp[:, nn * 512:(nn + 1) * 512], ksl, qsl,
                                 start=True, stop=True)
            pt = pt_pool.tile([128, 1024], bf16, tag="pt", name="pt")
            col = hl * 64 + b * 16 + kt
            nc.scalar.activation(pt[:], sp[:], Exp,
                                 bias=rb_sb[:, col:col + 1], scale=1.0)
            pts.append(pt)

        def pv_iter(b, hl, ctxp, pt, kt):
            """ctx^T += [V|1].T @ P^T for key tile kt (V stationary)."""
            vb = kt * 130 + hl * 65
            for nn in range(2):
                nc.tensor.matmul(ctxp[:, nn * 512:(nn + 1) * 512],
                                 vsbs[b][:, vb:vb + 65],
                                 pt[:, nn * 512:(nn + 1) * 512],
                                 start=(kt == 0), stop=(kt == SKT - 1))

        def norm_iter(hl, qh, ctxp, stage):
            """Transpose ctx^T back per query tile; divide by denominator."""
            pb = hl * 64
            ctxs = ctxs_pool.tile([65, QBLK], bf16, tag="cx", name="ctxs")
            nc.vector.tensor_copy(ctxs[:, 0:512], ctxp[:, 0:512])
            nc.vector.tensor_copy(ctxs[:, 512:1024], ctxp[:, 512:1024])
            for qt in range(NQT):
                ctp = tp_psum.tile([128, 65], bf16, tag="tp", name="ctp")
                nc.tensor.transpose(ctp[:], ctxs[:, qt * 128:(qt + 1) * 128],
                                    ident[0:65, 0:65])
                gq = qh * NQT + qt
                rcp = rcp_pool.tile([128, 1], f32, tag="rcp", name="rcp")
                nc.vector.reciprocal(rcp[:], ctp[:, 64:65])
                nc.vector.tensor_scalar_mul(
                    stage[:, gq * 128 + pb:gq * 128 + pb + 64],
                    ctp[:, 0:64], rcp[:, 0:1])

        def flush_out(bb):
            nc.sync.dma_start(
                outd[bb * S:(bb + 1) * S, :]
                .rearrange("(gq q) hd -> q gq hd", q=128),
                get_stage(bb)[:].rearrange("q (gq hd) -> q gq hd", hd=DPC))

        # ---- the schedule ----
        # preamble: K and Q projections of batch 0
        for s in range(NSUB):
            proj_k(0, s)
            proj_q(0, s, pop_xts=True)

        # proj slice units carried by each item (batch bb spread over
        # items 4(bb-1)+1..4(bb-1)+4; batch-0 V inside item 0)
        # k/q/v of one sub-chunk stay adjacent so the shared X tiles live
        # for at most two sub-chunks (xt pool rotation depth)
        units = {0: [lambda s=s: proj_v(0, s) for s in range(NSUB)]}
        for bb in range(1, B):
            base = 4 * (bb - 1)
            for s in range(NSUB):
                units[base + 1 + s] = [
                    lambda s=s, b=bb: proj_k(b, s),
                    lambda s=s, b=bb: proj_q(b, s),
                    lambda s=s, b=bb: proj_v(b, s)]

        seq = [(b, hl, qh) for b in range(B) for hl in range(HPC)
               for qh in range(2)]
        prev = None   # (b, hl, qh, pts)
        for idx, (b, hl, qh) in enumerate(seq):
            pts = []
            iu = units.get(idx, [])
            pctxp = None
            if prev is not None:
                pctxp = ctx_psum.tile([65, QBLK], f32, tag="ctx",
                                      name="pctxp")
            for kt in range(SKT):
                scores_exp(b, hl, qh, kt, pts)
                if prev is not None:
                    pv_iter(prev[0], prev[1], pctxp, prev[3][kt], kt)
                if kt in (3, 7, 11, 14) and iu:
                    iu.pop(0)()
            if prev is not None:
                norm_iter(prev[1], prev[2], pctxp, get_stage(prev[0]))
                if prev[0] != b:
                    flush_out(prev[0])
            prev = (b, hl, qh, pts)
        # tail: drain the final item
        pctxp = ctx_psum.tile([65, QBLK], f32, tag="ctx", name="pctxp")
        for kt in range(SKT):
            pv_iter(prev[0], prev[1], pctxp, prev[3][kt], kt)
        norm_iter(prev[1], prev[2], pctxp, get_stage(prev[0]))
        flush_out(prev[0])

    nc.compile()
    return nc


def get_program():
    if "nc" not in _cache:
        _cache["nc"] = _build_program()
    return _cache["nc"]


def make_in_maps(hidden_states, attention_mask, seg_ids, Wq, bq, Wk, Wv, bv,
                 seg_table, b_q_s):
    """Host-side shard + layout prep. Cheap (weights/bias reshapes, one bf16
    cast of X, 2-row segment gather); all O(T*S) math stays on device."""
    bf = ml_dtypes.bfloat16
    X = np.asarray(hidden_states, np.float32).reshape(T, HID)
    xb = np.ascontiguousarray(X.astype(bf).T)
    m = np.asarray(seg_ids).reshape(T).astype(np.int64)
    mask = np.asarray(attention_mask, np.float32).reshape(B, S)
    st = np.asarray(seg_table, np.float32)              # [2, HID]
    bqs = np.asarray(b_q_s, np.float32).reshape(NH, DH)
    Wq = np.asarray(Wq, np.float32)
    Wk = np.asarray(Wk, np.float32)
    Wv = np.asarray(Wv, np.float32)
    bq = np.asarray(bq, np.float32)
    bv = np.asarray(bv, np.float32)

    # mask-only per-key bias, same layout for both heads of a core:
    # rb[key, hl*64 + b*16 + kt] = mask[b, kt*128+key]
    rb_half = mask.reshape(B, 16, 128).transpose(2, 0, 1).reshape(128, 64)
    rb_c = np.ascontiguousarray(
        np.concatenate([rb_half, rb_half], axis=1).astype(np.float32))

    in_maps = []
    for c in range(N_CORES):
        h0, h1 = c * HPC, c * HPC + 1
        s0, s1 = slice(h0 * DH, (h0 + 1) * DH), slice(h1 * DH, (h1 + 1) * DH)
        # bias vectors: bqa = bq (scaled path applies (q+bq)*s),
        # bqb = bq + b_q_s (plain path)
        bqa_c = np.concatenate([bq[s0], bq[s1]])
        bqb_c = np.concatenate([bq[s0] + bqs[h0], bq[s1] + bqs[h1]])
        # segrep^T halves: [0:64]=head1, [64:128]=head0
        srt_c = np.empty((128, T), np.float32)
        srt_c[0:64, :] = st[np.ix_(m, range(s1.start, s1.stop))].T
        srt_c[64:128, :] = st[np.ix_(m, range(s0.start, s0.stop))].T
        sl = slice(c * DPC, (c + 1) * DPC)
        in_maps.append({
            "xb": xb,
            "wq": np.ascontiguousarray(Wq[sl, :].T).astype(bf),
            "wk": np.ascontiguousarray(Wk[sl, :].T).astype(bf),
            "wv": np.ascontiguousarray(Wv[sl, :].T).astype(bf),
            "srt": srt_c.astype(bf),
            "rb": rb_c,
            "bqa": np.ascontiguousarray(bqa_c.reshape(DPC, 1)),
            "bqb": np.ascontiguousarray(bqb_c.reshape(DPC, 1)),
            "bv": np.ascontiguousarray(bv[sl].reshape(DPC, 1)),
        })
    return in_maps


def assemble_output(results):
    return np.concatenate(
        [np.asarray(r["out"], np.float32).reshape(B, S, DPC) for r in results],
        axis=2)


def kernel(hidden_states, attention_mask, seg_ids, Wq, bq, Wk, Wv, bv,
           seg_table, b_q_s):
    from concourse.bass_utils import run_bass_kernel_spmd
    nc = get_program()
    in_maps = make_in_maps(hidden_states, attention_mask, seg_ids, Wq, bq,
                           Wk, Wv, bv, seg_table, b_q_s)
    res = run_bass_kernel_spmd(nc, in_maps, list(range(N_CORES)))
    return assemble_output(res.results)


if __name__ == "__main__":
    get_program()
    print("program built + compiled ok")


# revision 16
# speedup vs baseline: 1.1230x; 1.0127x over previous
"""BertSelfAttention (with segment-embedding score bias) on 8 trn2 NeuronCores.

Math (reference semantics), per head h:
    q = X @ Wq.T + bq ; k = X @ Wk.T ; v = X @ Wv.T + bv
    scores = (q*s) @ k.T + (q + b_q_s) @ segrep.T + mask ;  s = 1/sqrt(DH)
    out = softmax(scores) @ v

Device formulation: per (batch, head) an augmented 128-deep contraction
    qhat = [q*s ; q + b_q_s],  khat = [k ; segrep]
    scores^T = khat_tile.T @ qhat  (mask folded into the exp bias); exp on
ACT; PV computed V-stationary as ctx^T = [V|1].T @ P^T (the ones column
accumulates the softmax denominator), then a PE transpose per 128-query
tile and a reciprocal-scaled writeback.

Schedule: the ACT engine's exp stream (~1.04us per [128,1024] tile) is the
hard floor, so everything else hides in its shadow: PV of item g-1
interleaves per key-tile with the scores of item g, and the projections are
cut into 512-token single-matrix slices placed between score tiles so the
score/psum stream (and with it the exp pipeline) never pauses for a long
projection burst. ACT runs nothing but exp; scaling/bias work lives on DVE.

Sharding: tensor-parallel over heads; core c owns heads 2c, 2c+1. Each core
reads the full tokens and computes its head-slice of the output; the host
concatenates along the hidden dim. No collectives.
"""

import os
import sys

for _p in ("/opt/trn_rl_repo", "/root/.axon_site/_ro/trn_rl_repo"):
    if os.path.isdir(_p) and _p not in sys.path:
        sys.path.append(_p)

import numpy as np
import ml_dtypes

B, S, NH, DH = 4, 2048, 16, 64
HID = NH * DH          # 1024
T = B * S              # 8192
N_CORES = 8
HPC = NH // N_CORES    # heads per core = 2
DPC = HPC * DH         # out dims per core = 128
SCALE = 1.0 / 8.0      # 1/sqrt(DH)
KT = HID // 128        # 8 contraction tiles for the projections
SKT = S // 128         # 16 key tiles per sequence
PSUB = 1024            # projection slice tokens
NSUB = S // PSUB       # 4 slices per batch
QBLK = 1024            # queries per item
NQT = QBLK // 128      # query tiles per item

_cache = {}


def _build_program():
    import concourse.bacc as bacc
    import concourse.tile as tile
    from concourse import masks, mybir
    from contextlib import ExitStack

    bf16 = mybir.dt.bfloat16
    f32 = mybir.dt.float32
    Exp = mybir.ActivationFunctionType.Exp
    add = mybir.AluOpType.add
    mult = mybir.AluOpType.mult

    nc = bacc.Bacc("TRN2", target_bir_lowering=False, debug=False,
                   num_devices=N_CORES)
    xb = nc.dram_tensor("xb", [HID, T], bf16, kind="ExternalInput")
    wq = nc.dram_tensor("wq", [HID, DPC], bf16, kind="ExternalInput")
    wk = nc.dram_tensor("wk", [HID, DPC], bf16, kind="ExternalInput")
    wv = nc.dram_tensor("wv", [HID, DPC], bf16, kind="ExternalInput")
    srt = nc.dram_tensor("srt", [128, T], bf16, kind="ExternalInput")
    rb = nc.dram_tensor("rb", [128, 128], f32, kind="ExternalInput")
    bqa = nc.dram_tensor("bqa", [DPC, 1], f32, kind="ExternalInput")
    bqb = nc.dram_tensor("bqb", [DPC, 1], f32, kind="ExternalInput")
    bv = nc.dram_tensor("bv", [DPC, 1], f32, kind="ExternalInput")
    outd = nc.dram_tensor("out", [T, DPC], f32, kind="ExternalOutput")
    # cross-partition bounce for the plain-q half of qhat
    qbounce = nc.dram_tensor("qbounce", [2, B * NSUB, 64, PSUB], bf16)

    with tile.TileContext(nc) as tc, ExitStack() as octx:
        const = octx.enter_context(tc.tile_pool(name="const", bufs=1))
        kh_pool = octx.enter_context(tc.tile_pool(name="khp", bufs=4))
        qh_pool = octx.enter_context(tc.tile_pool(name="qhp", bufs=4))
        v_pool = octx.enter_context(tc.tile_pool(name="vp", bufs=3))
        stage_pool = octx.enter_context(tc.tile_pool(name="stp", bufs=2))
        pt_pool = octx.enter_context(tc.tile_pool(name="ptp", bufs=36))
        xt_pool = octx.enter_context(tc.tile_pool(name="xtp", bufs=16))
        qs_pool = octx.enter_context(tc.tile_pool(name="qsp", bufs=2))
        vt_pool = octx.enter_context(tc.tile_pool(name="vtp", bufs=2))
        ctxs_pool = octx.enter_context(tc.tile_pool(name="ctxsp", bufs=2))
        rcp_pool = octx.enter_context(tc.tile_pool(name="rcpp", bufs=8))
        sc_psum = octx.enter_context(
            tc.tile_pool(name="scp", bufs=2, space="PSUM"))
        ctx_psum = octx.enter_context(
            tc.tile_pool(name="ctxp", bufs=1, space="PSUM"))
        tp_psum = octx.enter_context(
            tc.tile_pool(name="tpp", bufs=2, space="PSUM"))

        # ---- constants ----
        rb_sb = const.tile([128, 128], f32)
        bqa_sb = const.tile([DPC, 1], f32)
        bqb_sb = const.tile([DPC, 1], f32)
        bv_sb = const.tile([DPC, 1], f32)
        ident = const.tile([128, 128], bf16)
        wq_sb = const.tile([128, KT, DPC], bf16)
        wk_sb = const.tile([128, KT, DPC], bf16)
        wv_sb = const.tile([128, KT, DPC], bf16)
        nc.sync.dma_start(rb_sb[:], rb[:])
        nc.sync.dma_start(bqa_sb[:], bqa[:])
        nc.sync.dma_start(bqb_sb[:], bqb[:])
        nc.sync.dma_start(bv_sb[:], bv[:])
        for w_sb, w in ((wq_sb, wq), (wk_sb, wk), (wv_sb, wv)):
            nc.sync.dma_start(w_sb[:],
                              w.rearrange("(kt p) d -> p kt d", p=128))
        masks.make_identity(nc, ident[:])

        # PE warmup: dense matmuls un-throttle the clock gate while the
        # first input DMAs are in flight.
        wup = sc_psum.tile([128, 1024], f32, tag="sc", name="wup")
        for _ in range(48):
            nc.tensor.matmul(wup[:, 0:128], ident[:], ident[:],
                             start=True, stop=True)

        # ---- per-batch resident tiles (rotating pools) ----
        khs, qhs, vsbs, stages = {}, {}, {}, {}

        def alloc_batch(bb):
            khs[bb] = [kh_pool.tile([128, S], bf16, tag="kh",
                                    name=f"kh{bb}_{i}") for i in range(2)]
            qhs[bb] = [qh_pool.tile([128, S], bf16, tag="qh",
                                    name=f"qh{bb}_{i}") for i in range(2)]
            v = v_pool.tile([128, SKT * 130], bf16, tag="vsb",
                            name=f"vsb{bb}")
            nc.gpsimd.memset(v[:], 1.0)   # preset denominators' ones cols
            vsbs[bb] = v

        def get_stage(bb):
            if bb not in stages:
                stages[bb] = stage_pool.tile([128, 16 * 128], f32, tag="st",
                                             name=f"st{bb}")
            return stages[bb]

        # ---- projection slices (one matrix x 512 tokens each) ----
        xts_cur = {}

        def proj_mm(psum_tile, w_sb, xts):
            for kt in range(KT):
                for nn in range(PSUB // 512):
                    nc.tensor.matmul(
                        psum_tile[:, nn * 512:(nn + 1) * 512],
                        w_sb[:, kt, :], xts[kt][:, nn * 512:(nn + 1) * 512],
                        start=(kt == 0), stop=(kt == KT - 1))

        def proj_k(bb, s):
            if bb not in khs:
                alloc_batch(bb)
            cs = slice(bb * S + s * PSUB, bb * S + (s + 1) * PSUB)
            lb = slice(s * PSUB, (s + 1) * PSUB)
            xts = []
            for kt in range(KT):
                xt = xt_pool.tile([128, PSUB], bf16, tag="xt", name="xt")
                nc.sync.dma_start(xt[:], xb[kt * 128:(kt + 1) * 128, cs])
                xts.append(xt)
            xts_cur[(bb, s)] = xts
            nc.sync.dma_start(khs[bb][0][64:128, lb], srt[64:128, cs])
            nc.sync.dma_start(khs[bb][1][0:64, lb], srt[0:64, cs])
            kp = sc_psum.tile([128, 1024], f32, tag="sc", name="kp")
            proj_mm(kp, wk_sb, xts)
            nc.vector.tensor_copy(khs[bb][0][0:64, lb], kp[0:64, 0:PSUB])
            nc.vector.tensor_copy(khs[bb][1][64:128, lb], kp[64:128, 0:PSUB])

        def proj_q(bb, s, pop_xts=False):
            lb = slice(s * PSUB, (s + 1) * PSUB)
            xts = xts_cur.pop((bb, s)) if pop_xts else xts_cur[(bb, s)]
            qp = sc_psum.tile([128, 1024], f32, tag="sc", name="qp")
            proj_mm(qp, wq_sb, xts)
            # scaled halves in-lane: out = (q + bq) * s  (bqa holds bq)
            nc.vector.tensor_scalar(qhs[bb][0][0:64, lb], qp[0:64, 0:PSUB],
                                    bqa_sb[0:64, 0:1], SCALE, add, mult)
            nc.vector.tensor_scalar(qhs[bb][1][64:128, lb],
                                    qp[64:128, 0:PSUB],
                                    bqa_sb[64:128, 0:1], SCALE, add, mult)
            # plain halves (q + bq + b_q_s) bounce through DRAM to flip lanes
            qs = qs_pool.tile([128, PSUB], bf16, tag="qs", name="qs")
            nc.vector.tensor_scalar_add(qs[:], qp[:, 0:PSUB], bqb_sb[:, 0:1])
            ci = bb * NSUB + s
            nc.sync.dma_start(qbounce[0, ci], qs[0:64, :])
            nc.sync.dma_start(qbounce[1, ci], qs[64:128, :])
            nc.sync.dma_start(qhs[bb][0][64:128, lb], qbounce[0, ci])
            nc.sync.dma_start(qhs[bb][1][0:64, lb], qbounce[1, ci])

        def proj_v(bb, s, fresh=False):
            xts = None if fresh else xts_cur.pop((bb, s))
            if xts is None:
                # batch-0 V runs later than its K/Q: re-DMA the X slice
                cs = slice(bb * S + s * PSUB, bb * S + (s + 1) * PSUB)
                xts = []
                for kt in range(KT):
                    xt = xt_pool.tile([128, PSUB], bf16, tag="xt", name="xt")
                    nc.sync.dma_start(xt[:], xb[kt * 128:(kt + 1) * 128, cs])
                    xts.append(xt)
            vp = sc_psum.tile([128, 1024], f32, tag="sc", name="vp")
            proj_mm(vp, wv_sb, xts)
            vt = vt_pool.tile([128, PSUB], bf16, tag="vt", name="vt")
            nc.vector.tensor_scalar_add(vt[:], vp[:, 0:PSUB], bv_sb[:, 0:1])
            for tt in range(PSUB // 128):
                gt = s * (PSUB // 128) + tt
                vtp = tp_psum.tile([128, 128], bf16, tag="tp", name="vtp")
                nc.tensor.transpose(
                    vtp[:], vt[:, tt * 128:(tt + 1) * 128], ident[:])
                nc.vector.tensor_copy(
                    vsbs[bb][:, gt * 130:(gt + 1) * 130]
                    .rearrange("p (h x) -> p h x", h=2)[:, :, 0:64],
                    vtp[:].rearrange("p (h d) -> p h d", h=2))

        # ---- attention pieces ----
        def scores_exp(b, hl, qh, kt, pts):
            sp = sc_psum.tile([128, 1024], f32, tag="sc", name="sp")
            ksl = khs[b][hl][:, kt * 128:(kt + 1) * 128]
            for nn in range(2):
                qsl = qhs[b][hl][:, qh * QBLK + nn * 512:
                                 qh * QBLK + (nn + 1) * 512]
                nc.tensor.matmul(sp[:, nn * 512:(nn + 1) * 512], ksl, qsl,
                                 start=True, stop=True)
            pt = pt_pool.tile([128, 1024], bf16, tag="pt", name="pt")
            col = hl * 64 + b * 16 + kt
            nc.scalar.activation(pt[:], sp[:], Exp,
                                 bias=rb_sb[:, col:col + 1], scale=1.0)
            pts.append(pt)

        def pv_iter(b, hl, ctxp, pt, kt):
            """ctx^T += [V|1].T @ P^T for key tile kt (V stationary)."""
            vb = kt * 130 + hl * 65
            for nn in range(2):
                nc.tensor.matmul(ctxp[:, nn * 512:(nn + 1) * 512],
                                 vsbs[b][:, vb:vb + 65],
                                 pt[:, nn * 512:(nn + 1) * 512],
                                 start=(kt == 0), stop=(kt == SKT - 1))

        def norm_iter(hl, qh, ctxp, stage):
            """Transpose ctx^T back per query tile; divide by denominator."""
            pb = hl * 64
            ctxs = ctxs_pool.tile([65, QBLK], bf16, tag="cx", name="ctxs")
            nc.vector.tensor_copy(ctxs[:, 0:512], ctxp[:, 0:512])
            nc.vector.tensor_copy(ctxs[:, 512:1024], ctxp[:, 512:1024])
            for qt in range(NQT):
                ctp = tp_psum.tile([128, 65], bf16, tag="tp", name="ctp")
                nc.tensor.transpose(ctp[:], ctxs[:, qt * 128:(qt + 1) * 128],
                                    ident[0:65, 0:65])
                gq = qh * NQT + qt
                rcp = rcp_pool.tile([128, 1], f32, tag="rcp", name="rcp")
                nc.vector.reciprocal(rcp[:], ctp[:, 64:65])
                nc.vector.tensor_scalar_mul(
                    stage[:, gq * 128 + pb:gq * 128 + pb + 64],
                    ctp[:, 0:64], rcp[:, 0:1])

        def flush_out(bb):
            nc.sync.dma_start(
                outd[bb * S:(bb + 1) * S, :]
                .rearrange("(gq q) hd -> q gq hd", q=128),
                get_stage(bb)[:].rearrange("q (gq hd) -> q gq hd", hd=DPC))

        # ---- the schedule ----
        # preamble: just K and Q of the first 1024 tokens of batch 0 -- the
        # minimum item (0,0,qh0) needs; everything else rides between score
        # tiles as in-item units in the shadow of the exp stream.
        proj_k(0, 0)
        proj_q(0, 0)

        # k/q of a slice stay adjacent so the shared X tiles live at most
        # two slices (xt pool rotation depth); batch-0 V re-DMAs its X.
        units = {
            0: [lambda: proj_k(0, 1), lambda: proj_q(0, 1),
                lambda: proj_v(0, 0, fresh=True)],
            1: [lambda: proj_v(0, 1, fresh=True), lambda: proj_k(1, 0)],
            2: [lambda: proj_q(1, 0), lambda: proj_v(1, 0)],
            3: [lambda: proj_k(1, 1), lambda: proj_q(1, 1)],
            4: [lambda: proj_v(1, 1)],
        }
        for bb in range(2, B):
            base = 4 * (bb - 1)
            units[base + 1] = [lambda b=bb: proj_k(b, 0),
                               lambda b=bb: proj_q(b, 0)]
            units[base + 2] = [lambda b=bb: proj_v(b, 0),
                               lambda b=bb: proj_k(b, 1)]
            units[base + 3] = [lambda b=bb: proj_q(b, 1)]
            units[base + 4] = [lambda b=bb: proj_v(b, 1)]

        seq = [(b, hl, qh) for b in range(B) for hl in range(HPC)
               for qh in range(2)]
        prev = None   # (b, hl, qh, pts)
        for idx, (b, hl, qh) in enumerate(seq):
            pts = []
            iu = units.get(idx, [])
            pctxp = None
            if prev is not None:
                pctxp = ctx_psum.tile([65, QBLK], f32, tag="ctx",
                                      name="pctxp")
            for kt in range(SKT):
                scores_exp(b, hl, qh, kt, pts)
                if prev is not None:
                    pv_iter(prev[0], prev[1], pctxp, prev[3][kt], kt)
                if kt in (3, 9, 13) and iu:
                    iu.pop(0)()
            if prev is not None:
                norm_iter(prev[1], prev[2], pctxp, get_stage(prev[0]))
                if prev[0] != b:
                    flush_out(prev[0])
            prev = (b, hl, qh, pts)
        # tail: drain the final item
        pctxp = ctx_psum.tile([65, QBLK], f32, tag="ctx", name="pctxp")
        for kt in range(SKT):
            pv_iter(prev[0], prev[1], pctxp, prev[3][kt], kt)
        norm_iter(prev[1], prev[2], pctxp, get_stage(prev[0]))
        flush_out(prev[0])

    nc.compile()
    return nc


def get_program():
    if "nc" not in _cache:
        _cache["nc"] = _build_program()
    return _cache["nc"]


def make_in_maps(hidden_states, attention_mask, seg_ids, Wq, bq, Wk, Wv, bv,
                 seg_table, b_q_s):
    """Host-side shard + layout prep. Cheap (weights/bias reshapes, one bf16
    cast of X, 2-row segment gather); all O(T*S) math stays on device."""
    bf = ml_dtypes.bfloat16
    X = np.asarray(hidden_states, np.float32).reshape(T, HID)
    xb = np.ascontiguousarray(X.astype(bf).T)
    m = np.asarray(seg_ids).reshape(T).astype(np.int64)
    mask = np.asarray(attention_mask, np.float32).reshape(B, S)
    st = np.asarray(seg_table, np.float32)              # [2, HID]
    bqs = np.asarray(b_q_s, np.float32).reshape(NH, DH)
    Wq = np.asarray(Wq, np.float32)
    Wk = np.asarray(Wk, np.float32)
    Wv = np.asarray(Wv, np.float32)
    bq = np.asarray(bq, np.float32)
    bv = np.asarray(bv, np.float32)

    # mask-only per-key bias, same layout for both heads of a core:
    # rb[key, hl*64 + b*16 + kt] = mask[b, kt*128+key]
    rb_half = mask.reshape(B, 16, 128).transpose(2, 0, 1).reshape(128, 64)
    rb_c = np.ascontiguousarray(
        np.concatenate([rb_half, rb_half], axis=1).astype(np.float32))

    in_maps = []
    for c in range(N_CORES):
        h0, h1 = c * HPC, c * HPC + 1
        s0, s1 = slice(h0 * DH, (h0 + 1) * DH), slice(h1 * DH, (h1 + 1) * DH)
        # bias vectors: bqa = bq (scaled path applies (q+bq)*s),
        # bqb = bq + b_q_s (plain path)
        bqa_c = np.concatenate([bq[s0], bq[s1]])
        bqb_c = np.concatenate([bq[s0] + bqs[h0], bq[s1] + bqs[h1]])
        # segrep^T halves: [0:64]=head1, [64:128]=head0
        srt_c = np.empty((128, T), np.float32)
        srt_c[0:64, :] = st[np.ix_(m, range(s1.start, s1.stop))].T
        srt_c[64:128, :] = st[np.ix_(m, range(s0.start, s0.stop))].T
        sl = slice(c * DPC, (c + 1) * DPC)
        in_maps.append({
            "xb": xb,
            "wq": np.ascontiguousarray(Wq[sl, :].T).astype(bf),
            "wk": np.ascontiguousarray(Wk[sl, :].T).astype(bf),
            "wv": np.ascontiguousarray(Wv[sl, :].T).astype(bf),
            "srt": srt_c.astype(bf),
            "rb": rb_c,
            "bqa": np.ascontiguousarray(bqa_c.reshape(DPC, 1)),
            "bqb": np.ascontiguousarray(bqb_c.reshape(DPC, 1)),
            "bv": np.ascontiguousarray(bv[sl].reshape(DPC, 1)),
        })
    return in_maps


def assemble_output(results):
    return np.concatenate(
        [np.asarray(r["out"], np.float32).reshape(B, S, DPC) for r in results],
        axis=2)


def kernel(hidden_states, attention_mask, seg_ids, Wq, bq, Wk, Wv, bv,
           seg_table, b_q_s):
    from concourse.bass_utils import run_bass_kernel_spmd
    nc = get_program()
    in_maps = make_in_maps(hidden_states, attention_mask, seg_ids, Wq, bq,
                           Wk, Wv, bv, seg_table, b_q_s)
    res = run_bass_kernel_spmd(nc, in_maps, list(range(N_CORES)))
    return assemble_output(res.results)


if __name__ == "__main__":
    get_program()
    print("program built + compiled ok")
